# revision 22
# baseline (speedup 1.0000x reference)
"""Causal multi-head self-attention on 8 TRN2 NeuronCores.

Sharding: batch (2) x head-groups (4) -> 8 cores. Each core computes the
qkv projection for its 4 heads of its batch, full causal attention for
those heads, and a partial output projection (its head slice of w_out);
the host sums the 4 partials per batch.

Per-core pipeline (v3):
  A) x^T arrives pre-transposed (bf16) from the host, so no PE
     transposes. Q^T/K^T (f32r in SBUF, head dims on partitions) via
     w^T-stationary bf16 matmuls; V natural ([s, d]) bf16 with a ones
     column per head (softmax denominator rides the PV matmul).
  B) attention in 256-wide q tiles: S^T = K^T.T @ Q^T (k on partitions,
     f32r), P = exp(S/8) on ACT into bf16, staircase mask on the two
     diagonal k-blocks (DVE), then O[q, d] += P_slice.T @ [V|1] with P as
     the 128-wide bf16 stationary. O is q-on-partitions, so the softmax
     denominator is a per-partition scalar: DVE reciprocal +
     tensor_scalar_mul, fused with the PSUM->SBUF bf16 copy. Two heads
     pack into one [128, 128] bf16 tile, PE-transposed into
     aoT2[hp] = [2 heads' dims, s].
  C) partial[s, :] = sum_hp aoT2[hp].T @ wo2[hp], staged bf16 and DMA'd;
     host converts/sums partials in f32.

Emission is software-pipelined and deficit-paced: each group's PV is one
unit late (PE runs ahead of ACT's exp); projection chains for s-tile
st+1, deferred O-transposes, and output-projection chunks are queued as
PE fillers and dispensed whenever the running ACT-vs-PE budget goes
negative, so PE never starves while ACT crunches exp. Leftover fillers
(mostly phase C) drain at the end, covering ACT's causal-tail overrun.
"""

import math
import numpy as np

import concourse.bacc as bacc
import concourse.mybir as mybir
import concourse.tile as tile
from concourse.masks import make_identity
from concourse.bass_utils import run_bass_kernel_spmd

F32 = mybir.dt.float32
F32R = mybir.dt.float32r
BF16 = mybir.dt.bfloat16
EXP = mybir.ActivationFunctionType.Exp

D_MODEL = 1024
HEAD_DIM = 64
B, S = 2, 2048
N_CORES = 8
OLOC = 256                  # 4 heads x 64 dims per core
SCALE = 1.0 / math.sqrt(HEAD_DIM)
G = 4                       # k-blocks (128 each) per S/exp group

PE_NS = 1.0 / 2.4           # ns per PE cycle at full p-state
ACT_NS = 1.0 / 1.2

_CACHE = {}


def build_nc():
    nc = bacc.Bacc("TRN2", target_bir_lowering=False, debug=False)

    x_d = nc.dram_tensor("x_t", [D_MODEL, S], BF16, kind="ExternalInput")
    wqk_d = nc.dram_tensor("wqk_t", [D_MODEL, 512], BF16, kind="ExternalInput")
    wv_d = nc.dram_tensor("wv_t", [D_MODEL, OLOC], BF16, kind="ExternalInput")
    wo_d = nc.dram_tensor("wo_t", [OLOC, D_MODEL], BF16, kind="ExternalInput")
    out_d = nc.dram_tensor("out", [S, D_MODEL], BF16, kind="ExternalOutput")

    with tile.TileContext(nc) as tc:
        with (
            tc.tile_pool(name="persist", bufs=1) as pp,
            tc.tile_pool(name="work", bufs=2) as wp,
            tc.tile_pool(name="psum", bufs=1, space="PSUM") as psp,
        ):
            ident = pp.tile([128, 128], BF16)
            make_identity(nc, ident[:])

            # staircase causal mask for the 2-block diagonal band of a
            # 256-wide q tile: M[p, c] = 1 iff p <= c - 128. Slice
            # [:, (1-j)*128:][:256] masks diagonal sub-block j.
            mask = pp.tile([128, 384], BF16)
            nc.gpsimd.memset(mask[:], 1.0)
            nc.gpsimd.affine_select(
                out=mask[:], in_=mask[:],
                compare_op=mybir.AluOpType.is_ge,
                fill=0.0, base=-128,
                pattern=[[1, 384]], channel_multiplier=-1,
            )

            xT = pp.tile([128, 8, S], BF16)
            wqk = pp.tile([128, 8, 512], BF16)
            wv = pp.tile([128, 8, OLOC], BF16)
            wo2 = pp.tile([128, 2, D_MODEL], BF16)
            qkT = [pp.tile([128, S], F32R, name=f"qkT{i}") for i in range(4)]
            v_sb = [pp.tile([128, 4, 65], BF16, name=f"v{j}")
                    for j in range(S // 128)]
            aoT2 = [pp.tile([128, S], BF16, name=f"aoT{i}") for i in range(2)]

            for j in range(S // 128):
                nc.gpsimd.memset(v_sb[j][:, :, 64:65], 1.0)

            # input DMAs (all SP queue), ordered so the first s-tile's
            # operands land first: wqk halves + st0 x slices, then wv,
            # then coarser x chunks for s 512.. plus wo.
            nc.sync.dma_start(wqk[:, 0, :], wqk_d[0:128, :])
            nc.sync.dma_start(xT[:, 0, 0:512], x_d[0:128, 0:512])
            nc.sync.dma_start(
                wqk[:, 1:4, :],
                wqk_d[128:512, :].rearrange("(c p) o -> p c o", p=128))
            nc.sync.dma_start(xT[:, 1, 0:512], x_d[128:256, 0:512])
            nc.sync.dma_start(
                wqk[:, 4:8, :],
                wqk_d[512:1024, :].rearrange("(c p) o -> p c o", p=128))
            for it in range(2, 8):
                nc.sync.dma_start(
                    xT[:, it, 0:512],
                    x_d[it * 128:(it + 1) * 128, 0:512])
            nc.sync.dma_start(
                wv[:],
                wv_d[:, :].rearrange("(c p) o -> p c o", p=128))
            for it in range(8):
                nc.sync.dma_start(
                    xT[:, it, 512:2048],
                    x_d[it * 128:(it + 1) * 128, 512:2048])
            nc.sync.dma_start(
                wo2[:],
                wo_d[:, :].rearrange("(hp p) o -> p hp o", p=128))

            # ---- pacing scheduler ----
            # Attention units stream continuously (their real pace is set
            # at runtime by ACT's exp throughput via the 2-deep pst
            # rotation); projection chains for later s-tiles and the
            # deferred transpose / output-projection work are dispensed
            # between units whenever the running (PE - ACT) bank dips
            # below a margin, so PE always has work while ACT crunches
            # and ACT never waits on a long dense PE block. Gates keep a
            # chain from being emitted before its x DMA plausibly landed
            # (a premature chain would block the in-order PE stream).
            sched = {"bank": 3000.0, "ucount": 0}
            MARGIN = 1500.0
            proj_qs = {1: [], 2: [], 3: []}   # st -> chains
            UNIT_GATE = {1: 2, 2: 10, 3: 28}  # min ucount to dispense
            late_q = []          # transposes + phase C (no deadline)

            def next_proj():
                for st in (1, 2, 3):
                    if proj_qs[st] and UNIT_GATE[st] <= sched["ucount"]:
                        return proj_qs[st]
                return None

            def charge(pe_ns, act_ns):
                sched["bank"] += pe_ns - act_ns
                sched["ucount"] += 1
                while sched["bank"] < MARGIN:
                    q = next_proj()
                    if q is None:
                        break
                    pe_ns2, fn = q.pop(0)
                    fn()
                    sched["bank"] += pe_ns2
                while sched["bank"] < 0.0 and late_q:
                    pe_ns2, fn = late_q.pop(0)
                    fn()
                    sched["bank"] += pe_ns2

            def flush_proj(st):
                while proj_qs[st]:
                    _, fn = proj_qs[st].pop(0)
                    fn()

            # ---- phase A: Q^T/K^T + V for one 512-wide s tile ----
            def qk_chain(st, ob):
                pqk = psp.tile([128, 512], F32, tag="mm", bufs=2, name="pqk")
                for it in range(8):
                    nc.tensor.matmul(
                        pqk[:],
                        wqk[:, it, ob * 128:(ob + 1) * 128],
                        xT[:, it, st * 512:(st + 1) * 512],
                        start=(it == 0), stop=(it == 7),
                        skip_group_check=True)
                nc.vector.tensor_copy(
                    qkT[ob][:, st * 512:(st + 1) * 512], pqk[:])

            def v_chain(st, j):
                pv = psp.tile([128, OLOC], F32, tag="mm", bufs=2, name="pv")
                s0 = st * 512 + j * 128
                for it in range(8):
                    nc.tensor.matmul(
                        pv[:],
                        xT[:, it, s0:s0 + 128],
                        wv[:, it, :],
                        start=(it == 0), stop=(it == 7),
                        skip_group_check=True)
                vt = v_sb[st * 4 + j]
                nc.vector.tensor_copy(
                    vt[:, :, 0:64],
                    pv[:].rearrange("p (h d) -> p h d", h=4))

            def queue_phaseA(st):
                for ob in range(4):
                    proj_qs[st].append((8 * 512 * PE_NS,
                                        lambda st=st, ob=ob:
                                        qk_chain(st, ob)))
                for j in range(4):
                    proj_qs[st].append((8 * 256 * PE_NS,
                                        lambda st=st, j=j: v_chain(st, j)))

            # ---- phase C: one 128-row output chunk ----
            osb_tiles = {}

            tail_mode = [False]   # end flush: ACT is idle, use it for copies

            def c_chunk(qt, sb, ob):
                s0 = qt * 256 + sb * 128
                pout = psp.tile([128, 512], F32, tag="mm", bufs=2,
                                name="pout")
                for hp in range(2):
                    nc.tensor.matmul(
                        pout[:],
                        aoT2[hp][:, s0:s0 + 128],
                        wo2[:, hp, ob * 512:(ob + 1) * 512],
                        start=(hp == 0), stop=(hp == 1),
                        skip_group_check=True)
                if ob == 0:
                    osb_tiles[qt, sb] = wp.tile([128, 2, 512], BF16,
                                                tag="osb", bufs=3,
                                                name="osb")
                osb = osb_tiles[qt, sb]
                if tail_mode[0] and ob == 0:
                    nc.scalar.copy(osb[:, ob, :], pout[:])
                else:
                    nc.vector.tensor_copy(osb[:, ob, :], pout[:])
                if ob == 1:
                    nc.sync.dma_start(out_d[s0:s0 + 128, :],
                                      osb_tiles.pop((qt, sb))[:])

            def queue_phaseC(qt):
                for sb in range(2):
                    for ob in range(2):
                        late_q.append(
                            (2 * 512 * PE_NS,
                             lambda qt=qt, sb=sb, ob=ob: c_chunk(qt, sb, ob)))

            # ---- phase B: attention with lag-2 PV emission ----
            # PV of group g is emitted two units late so the exp+mask
            # chain latency (~1us across ACT/DVE) is fully hidden behind
            # later S groups and dispensed filler work.
            pending = []

            def run_unit(s_fn, pv_fn, posts, pe_ns, act_ns):
                s_fn()
                charge(pe_ns, act_ns)
                pending.append((pv_fn, posts))
                if len(pending) > 2:
                    pv, ps = pending.pop(0)
                    pv()
                    for p in ps:
                        p()

            def flush_pending():
                while pending:
                    pv, ps = pending.pop(0)
                    pv()
                    for p in ps:
                        p()

            o2_tiles = {}

            def emit_head(qt, hp, hh):
                h = 2 * hp + hh
                nkb = 2 * (qt + 1)
                r0 = (h % 2) * 64
                q_t = qkT[h // 2]
                k_t = qkT[2 + h // 2]
                state = {}
                ngrp = (nkb + G - 1) // G

                def s_fn(kb0, g):
                    pst = psp.tile([128, G, 256], F32, tag="pst", bufs=2,
                                   name="pst")
                    for u in range(g):
                        kb = kb0 + u
                        nc.tensor.matmul(
                            pst[:, u, :],
                            k_t[r0:r0 + 64, kb * 128:(kb + 1) * 128],
                            q_t[r0:r0 + 64, qt * 256:(qt + 1) * 256],
                            start=True, stop=True)
                    p_t = wp.tile([128, G, 256], BF16, tag="p_t", bufs=4,
                                  name="p_t")
                    nc.scalar.activation(p_t[:, 0:g, :], pst[:, 0:g, :],
                                         EXP, scale=SCALE)
                    for u in range(g):
                        j = kb0 + u - (nkb - 2)
                        if j >= 0:  # diagonal band: staircase mask
                            nc.vector.tensor_mul(
                                p_t[:, u, :], p_t[:, u, :],
                                mask[:, (1 - j) * 128:(1 - j) * 128 + 256])
                    state["p_t", kb0] = p_t

                def pv_fn(kb0, g):
                    if kb0 == 0:
                        # one PSUM bank per open accumulation group: a
                        # second group's start in the same bank wipes the
                        # first group's partials
                        state["po"] = [
                            psp.tile([128, 65], F32, tag="po", bufs=2,
                                     name="po")
                            for _ in range(2)]
                    po = state["po"]
                    p_t = state.pop(("p_t", kb0))
                    for u in range(g):
                        kb = kb0 + u
                        for q2 in range(2):
                            nc.tensor.matmul(
                                po[q2][:],
                                p_t[:, u, q2 * 128:(q2 + 1) * 128],
                                v_sb[kb][:, h, :],
                                start=(kb == 0), stop=(kb == nkb - 1),
                                skip_group_check=True)

                def norm_fn():
                    po = state["po"]
                    if hh == 0:
                        o2_tiles[qt, hp] = [
                            wp.tile([128, 128], BF16, tag="o2", bufs=8,
                                    name="o2")
                            for _ in range(2)]
                    o2 = o2_tiles[qt, hp]
                    for q2 in range(2):
                        recip = wp.tile([128, 1], F32, tag="recip", bufs=2,
                                        name="recip")
                        nc.vector.reciprocal(recip[:], po[q2][:, 64:65])
                        nc.vector.tensor_scalar_mul(
                            o2[q2][:, hh * 64:(hh + 1) * 64],
                            po[q2][:, 0:64], recip[:])

                def trans_fn():
                    o2 = o2_tiles.pop((qt, hp))
                    for q2 in range(2):
                        ptr = psp.tile([128, 128], BF16, tag="mm", bufs=2,
                                       name="ptr")
                        nc.tensor.matmul(ptr[:], o2[q2][:], ident[:],
                                         is_transpose=True,
                                         skip_group_check=True)
                        nc.vector.tensor_copy(
                            aoT2[hp][:, qt * 256 + q2 * 128:
                                     qt * 256 + (q2 + 1) * 128], ptr[:])

                def queue_tail():
                    late_q.append((2 * 128 * PE_NS, trans_fn))
                    if hp == 1:
                        queue_phaseC(qt)

                prev_g = [0]
                for gi in range(ngrp):
                    kb0 = gi * G
                    g = min(G, nkb - kb0)
                    posts = []
                    if gi == ngrp - 1:
                        posts.append(norm_fn)
                        if hh == 1:
                            posts.append(queue_tail)
                    pe_ns = (256 * g + 130 * prev_g[0]) * PE_NS
                    act_ns = (256 * g + 222) * ACT_NS + 32
                    run_unit(lambda kb0=kb0, g=g: s_fn(kb0, g),
                             lambda kb0=kb0, g=g: pv_fn(kb0, g),
                             posts, pe_ns, act_ns)
                    prev_g[0] = g

            # ---- schedule ----
            # st0's own chains are hand-placed around qt0/qt1's units
            # (dispensing one after its consumer would deadlock the
            # in-order PE stream); later tiles' chains flow through the
            # gated filler queues, force-flushed before their first
            # consumer pair.
            for st in (1, 2, 3):
                queue_phaseA(st)

            qk_chain(0, 0)
            qk_chain(0, 2)
            emit_head(0, 0, 0)
            emit_head(0, 0, 1)
            qk_chain(0, 1)
            qk_chain(0, 3)
            v_chain(0, 0)
            v_chain(0, 1)
            emit_head(0, 1, 0)
            emit_head(0, 1, 1)
            emit_head(1, 0, 0)
            v_chain(0, 2)
            v_chain(0, 3)
            emit_head(1, 0, 1)
            emit_head(1, 1, 0)
            emit_head(1, 1, 1)

            for st in (1, 2, 3):
                flush_proj(st)
                for qt in (2 * st, 2 * st + 1):
                    for hp in range(2):
                        for hh in range(2):
                            emit_head(qt, hp, hh)
            flush_pending()
            tail_mode[0] = True
            while late_q:
                _, fn = late_q.pop(0)
                fn()

    nc.compile()
    return nc


def make_in_maps(x, w_qkv, w_out):
    import ml_dtypes
    bf = ml_dtypes.bfloat16
    in_maps = []
    for c in range(N_CORES):
        b, g = divmod(c, 4)
        wq = w_qkv[g * OLOC:(g + 1) * OLOC, :]
        wk = w_qkv[D_MODEL + g * OLOC:D_MODEL + (g + 1) * OLOC, :]
        wvs = w_qkv[2 * D_MODEL + g * OLOC:2 * D_MODEL + (g + 1) * OLOC, :]
        in_maps.append({
            "x_t": np.ascontiguousarray(x[b].T).astype(bf),
            "wqk_t": np.ascontiguousarray(
                np.concatenate([wq, wk], axis=0).T).astype(bf),
            "wv_t": np.ascontiguousarray(wvs.T).astype(bf),
            "wo_t": np.ascontiguousarray(
                w_out[:, g * OLOC:(g + 1) * OLOC].T).astype(bf),
        })
    return in_maps


def kernel(x, w_qkv, w_out):
    x = np.asarray(x, dtype=np.float32)
    w_qkv = np.asarray(w_qkv, dtype=np.float32)
    w_out = np.asarray(w_out, dtype=np.float32)

    if "nc" not in _CACHE:
        _CACHE["nc"] = build_nc()
    nc = _CACHE["nc"]

    in_maps = make_in_maps(x, w_qkv, w_out)
    _CACHE["in_maps"] = in_maps

    res = run_bass_kernel_spmd(nc, in_maps, list(range(N_CORES)))
    out = np.zeros((B, S, D_MODEL), dtype=np.float32)
    for c in range(N_CORES):
        out[c // 4] += np.asarray(res.results[c]["out"], dtype=np.float32)
    return out


# revision 23
# speedup vs baseline: 18458.3785x; 18458.3785x over previous
"""Causal multi-head self-attention on 8 TRN2 NeuronCores.

Sharding: batch (2) x head-groups (4) -> 8 cores. Each core computes the
qkv projection for its 4 heads of its batch, full causal attention for
those heads, and a partial output projection (its head slice of w_out);
the host sums the 4 partials per batch.

Per-core pipeline (v3):
  A) x^T arrives pre-transposed (bf16) from the host, so no PE
     transposes. Q^T/K^T (f32r in SBUF, head dims on partitions) via
     w^T-stationary bf16 matmuls; V natural ([s, d]) bf16 with a ones
     column per head (softmax denominator rides the PV matmul).
  B) attention in 256-wide q tiles: S^T = K^T.T @ Q^T (k on partitions,
     f32r), P = exp(S/8) on ACT into bf16, staircase mask on the two
     diagonal k-blocks (DVE), then O[q, d] += P_slice.T @ [V|1] with P as
     the 128-wide bf16 stationary. O is q-on-partitions, so the softmax
     denominator is a per-partition scalar: DVE reciprocal +
     tensor_scalar_mul, fused with the PSUM->SBUF bf16 copy. Two heads
     pack into one [128, 128] bf16 tile, PE-transposed into
     aoT2[hp] = [2 heads' dims, s].
  C) partial[s, :] = sum_hp aoT2[hp].T @ wo2[hp], staged bf16 and DMA'd;
     host converts/sums partials in f32.

Emission is software-pipelined and deficit-paced: each group's PV is one
unit late (PE runs ahead of ACT's exp); projection chains for s-tile
st+1, deferred O-transposes, and output-projection chunks are queued as
PE fillers and dispensed whenever the running ACT-vs-PE budget goes
negative, so PE never starves while ACT crunches exp. Leftover fillers
(mostly phase C) drain at the end, covering ACT's causal-tail overrun.
"""

import math
import numpy as np

import concourse.bacc as bacc
import concourse.mybir as mybir
import concourse.tile as tile
from concourse.masks import make_identity
from concourse.bass_utils import run_bass_kernel_spmd

F32 = mybir.dt.float32
F32R = mybir.dt.float32r
BF16 = mybir.dt.bfloat16
EXP = mybir.ActivationFunctionType.Exp

D_MODEL = 1024
HEAD_DIM = 64
B, S = 2, 2048
N_CORES = 8
OLOC = 256                  # 4 heads x 64 dims per core
SCALE = 1.0 / math.sqrt(HEAD_DIM)
G = 4                       # k-blocks (128 each) per S/exp group

PE_NS = 1.0 / 2.4           # ns per PE cycle at full p-state
ACT_NS = 1.0 / 1.2

_CACHE = {}


def build_nc():
    nc = bacc.Bacc("TRN2", target_bir_lowering=False, debug=False)

    x_d = nc.dram_tensor("x_t", [D_MODEL, S], BF16, kind="ExternalInput")
    wqk_d = nc.dram_tensor("wqk_t", [D_MODEL, 512], BF16, kind="ExternalInput")
    wv_d = nc.dram_tensor("wv_t", [D_MODEL, OLOC], BF16, kind="ExternalInput")
    wo_d = nc.dram_tensor("wo_t", [OLOC, D_MODEL], BF16, kind="ExternalInput")
    out_d = nc.dram_tensor("out", [S, D_MODEL], BF16, kind="ExternalOutput")

    with tile.TileContext(nc) as tc:
        with (
            tc.tile_pool(name="persist", bufs=1) as pp,
            tc.tile_pool(name="work", bufs=2) as wp,
            tc.tile_pool(name="psum", bufs=1, space="PSUM") as psp,
        ):
            ident = pp.tile([128, 128], BF16)
            make_identity(nc, ident[:])

            # staircase causal mask for the 2-block diagonal band of a
            # 256-wide q tile: M[p, c] = 1 iff p <= c - 128. Slice
            # [:, (1-j)*128:][:256] masks diagonal sub-block j.
            mask = pp.tile([128, 384], BF16)
            nc.gpsimd.memset(mask[:], 1.0)
            nc.gpsimd.affine_select(
                out=mask[:], in_=mask[:],
                compare_op=mybir.AluOpType.is_ge,
                fill=0.0, base=-128,
                pattern=[[1, 384]], channel_multiplier=-1,
            )

            xT = pp.tile([128, 8, S], BF16)
            wqk = pp.tile([128, 8, 512], BF16)
            wv = pp.tile([128, 8, OLOC], BF16)
            wo2 = pp.tile([128, 2, D_MODEL], BF16)
            qkT = [pp.tile([128, S], F32R, name=f"qkT{i}") for i in range(4)]
            v_sb = [pp.tile([128, 4, 65], BF16, name=f"v{j}")
                    for j in range(S // 128)]
            aoT2 = [pp.tile([128, S], BF16, name=f"aoT{i}") for i in range(2)]

            for j in range(S // 128):
                nc.gpsimd.memset(v_sb[j][:, :, 64:65], 1.0)

            # input DMAs (all SP queue), ordered so the first s-tile's
            # operands land first: wqk halves + st0 x slices, then wv,
            # then coarser x chunks for s 512.. plus wo.
            nc.sync.dma_start(wqk[:, 0, :], wqk_d[0:128, :])
            nc.sync.dma_start(xT[:, 0, 0:512], x_d[0:128, 0:512])
            nc.sync.dma_start(
                wqk[:, 1:4, :],
                wqk_d[128:512, :].rearrange("(c p) o -> p c o", p=128))
            nc.sync.dma_start(xT[:, 1, 0:512], x_d[128:256, 0:512])
            nc.sync.dma_start(
                wqk[:, 4:8, :],
                wqk_d[512:1024, :].rearrange("(c p) o -> p c o", p=128))
            for it in range(2, 8):
                nc.sync.dma_start(
                    xT[:, it, 0:512],
                    x_d[it * 128:(it + 1) * 128, 0:512])
            nc.sync.dma_start(
                wv[:],
                wv_d[:, :].rearrange("(c p) o -> p c o", p=128))
            for it in range(8):
                nc.sync.dma_start(
                    xT[:, it, 512:2048],
                    x_d[it * 128:(it + 1) * 128, 512:2048])
            nc.sync.dma_start(
                wo2[:],
                wo_d[:, :].rearrange("(hp p) o -> p hp o", p=128))

            # ---- pacing scheduler ----
            # Attention units stream continuously (their real pace is set
            # at runtime by ACT's exp throughput via the 2-deep pst
            # rotation); projection chains for later s-tiles and the
            # deferred transpose / output-projection work are dispensed
            # between units whenever the running (PE - ACT) bank dips
            # below a margin, so PE always has work while ACT crunches
            # and ACT never waits on a long dense PE block. Gates keep a
            # chain from being emitted before its x DMA plausibly landed
            # (a premature chain would block the in-order PE stream).
            sched = {"bank": 3000.0, "ucount": 0}
            MARGIN = 1500.0
            proj_qs = {1: [], 2: [], 3: []}   # st -> chains
            UNIT_GATE = {1: 2, 2: 8, 3: 8}    # min ucount to dispense
            late_q = []          # transposes + phase C (no deadline)

            def next_proj():
                for st in (1, 2, 3):
                    if proj_qs[st] and UNIT_GATE[st] <= sched["ucount"]:
                        return proj_qs[st]
                return None

            def charge(pe_ns, act_ns):
                sched["bank"] += pe_ns - act_ns
                sched["ucount"] += 1
                while sched["bank"] < MARGIN:
                    q = next_proj()
                    if q is None:
                        break
                    pe_ns2, fn = q.pop(0)
                    fn()
                    sched["bank"] += pe_ns2
                # keep phase C in reserve for the ACT-bound causal tail:
                # only draw it down once every projection chain is out
                if not (proj_qs[1] or proj_qs[2] or proj_qs[3]):
                    while sched["bank"] < 0.0 and late_q:
                        pe_ns2, fn = late_q.pop(0)
                        fn()
                        sched["bank"] += pe_ns2

            def flush_proj(st):
                while proj_qs[st]:
                    _, fn = proj_qs[st].pop(0)
                    fn()

            # ---- phase A: Q^T/K^T + V for one 512-wide s tile ----
            def qk_chain(st, ob):
                pqk = psp.tile([128, 512], F32, tag="mm", bufs=2, name="pqk")
                for it in range(8):
                    nc.tensor.matmul(
                        pqk[:],
                        wqk[:, it, ob * 128:(ob + 1) * 128],
                        xT[:, it, st * 512:(st + 1) * 512],
                        start=(it == 0), stop=(it == 7),
                        skip_group_check=True)
                nc.vector.tensor_copy(
                    qkT[ob][:, st * 512:(st + 1) * 512], pqk[:])

            def v_chain(st, j):
                pv = psp.tile([128, OLOC], F32, tag="mm", bufs=2, name="pv")
                s0 = st * 512 + j * 128
                for it in range(8):
                    nc.tensor.matmul(
                        pv[:],
                        xT[:, it, s0:s0 + 128],
                        wv[:, it, :],
                        start=(it == 0), stop=(it == 7),
                        skip_group_check=True)
                vt = v_sb[st * 4 + j]
                nc.vector.tensor_copy(
                    vt[:, :, 0:64],
                    pv[:].rearrange("p (h d) -> p h d", h=4))

            def queue_phaseA(st):
                for ob in range(4):
                    proj_qs[st].append((8 * 512 * PE_NS,
                                        lambda st=st, ob=ob:
                                        qk_chain(st, ob)))
                for j in range(4):
                    proj_qs[st].append((8 * 256 * PE_NS,
                                        lambda st=st, j=j: v_chain(st, j)))

            # ---- phase C: one 128-row output chunk ----
            osb_tiles = {}

            tail_mode = [False]   # end flush: ACT is idle, use it for copies

            def c_chunk(qt, sb, ob):
                s0 = qt * 256 + sb * 128
                pout = psp.tile([128, 512], F32, tag="mm", bufs=2,
                                name="pout")
                for hp in range(2):
                    nc.tensor.matmul(
                        pout[:],
                        aoT2[hp][:, s0:s0 + 128],
                        wo2[:, hp, ob * 512:(ob + 1) * 512],
                        start=(hp == 0), stop=(hp == 1),
                        skip_group_check=True)
                if ob == 0:
                    osb_tiles[qt, sb] = wp.tile([128, 2, 512], BF16,
                                                tag="osb", bufs=3,
                                                name="osb")
                osb = osb_tiles[qt, sb]
                if tail_mode[0] and ob == 0:
                    nc.scalar.copy(osb[:, ob, :], pout[:])
                else:
                    nc.vector.tensor_copy(osb[:, ob, :], pout[:])
                if ob == 1:
                    nc.sync.dma_start(out_d[s0:s0 + 128, :],
                                      osb_tiles.pop((qt, sb))[:])

            def queue_phaseC(qt):
                for sb in range(2):
                    for ob in range(2):
                        late_q.append(
                            (2 * 512 * PE_NS,
                             lambda qt=qt, sb=sb, ob=ob: c_chunk(qt, sb, ob)))

            # ---- phase B: attention with lag-2 PV emission ----
            # PV of group g is emitted two units late so the exp+mask
            # chain latency (~1us across ACT/DVE) is fully hidden behind
            # later S groups and dispensed filler work.
            pending = []

            def run_unit(s_fn, pv_fn, posts, pe_ns, act_ns):
                s_fn()
                charge(pe_ns, act_ns)
                pending.append((pv_fn, posts))
                if len(pending) > 2:
                    pv, ps = pending.pop(0)
                    pv()
                    for p in ps:
                        p()

            def flush_pending():
                while pending:
                    pv, ps = pending.pop(0)
                    pv()
                    for p in ps:
                        p()

            o2_tiles = {}

            def emit_head(qt, hp, hh):
                h = 2 * hp + hh
                nkb = 2 * (qt + 1)
                r0 = (h % 2) * 64
                q_t = qkT[h // 2]
                k_t = qkT[2 + h // 2]
                state = {}
                ngrp = (nkb + G - 1) // G

                def s_fn(kb0, g):
                    pst = psp.tile([128, G, 256], F32, tag="pst", bufs=2,
                                   name="pst")
                    for u in range(g):
                        kb = kb0 + u
                        nc.tensor.matmul(
                            pst[:, u, :],
                            k_t[r0:r0 + 64, kb * 128:(kb + 1) * 128],
                            q_t[r0:r0 + 64, qt * 256:(qt + 1) * 256],
                            start=True, stop=True)
                    p_t = wp.tile([128, G, 256], BF16, tag="p_t", bufs=4,
                                  name="p_t")
                    nc.scalar.activation(p_t[:, 0:g, :], pst[:, 0:g, :],
                                         EXP, scale=SCALE)
                    for u in range(g):
                        j = kb0 + u - (nkb - 2)
                        if j >= 0:  # diagonal band: staircase mask
                            nc.vector.tensor_mul(
                                p_t[:, u, :], p_t[:, u, :],
                                mask[:, (1 - j) * 128:(1 - j) * 128 + 256])
                    state["p_t", kb0] = p_t

                def pv_fn(kb0, g):
                    if kb0 == 0:
                        # one PSUM bank per open accumulation group: a
                        # second group's start in the same bank wipes the
                        # first group's partials
                        state["po"] = [
                            psp.tile([128, 65], F32, tag="po", bufs=2,
                                     name="po")
                            for _ in range(2)]
                    po = state["po"]
                    p_t = state.pop(("p_t", kb0))
                    for u in range(g):
                        kb = kb0 + u
                        for q2 in range(2):
                            nc.tensor.matmul(
                                po[q2][:],
                                p_t[:, u, q2 * 128:(q2 + 1) * 128],
                                v_sb[kb][:, h, :],
                                start=(kb == 0), stop=(kb == nkb - 1),
                                skip_group_check=True)

                def norm_fn():
                    po = state["po"]
                    if hh == 0:
                        o2_tiles[qt, hp] = [
                            wp.tile([128, 128], BF16, tag="o2", bufs=8,
                                    name="o2")
                            for _ in range(2)]
                    o2 = o2_tiles[qt, hp]
                    for q2 in range(2):
                        recip = wp.tile([128, 1], F32, tag="recip", bufs=2,
                                        name="recip")
                        nc.vector.reciprocal(recip[:], po[q2][:, 64:65])
                        nc.vector.tensor_scalar_mul(
                            o2[q2][:, hh * 64:(hh + 1) * 64],
                            po[q2][:, 0:64], recip[:])

                def trans_fn():
                    o2 = o2_tiles.pop((qt, hp))
                    for q2 in range(2):
                        ptr = psp.tile([128, 128], BF16, tag="mm", bufs=2,
                                       name="ptr")
                        nc.tensor.matmul(ptr[:], o2[q2][:], ident[:],
                                         is_transpose=True,
                                         skip_group_check=True)
                        nc.vector.tensor_copy(
                            aoT2[hp][:, qt * 256 + q2 * 128:
                                     qt * 256 + (q2 + 1) * 128], ptr[:])

                def queue_tail():
                    late_q.append((2 * 128 * PE_NS, trans_fn))
                    if hp == 1:
                        queue_phaseC(qt)

                prev_g = [0]
                for gi in range(ngrp):
                    kb0 = gi * G
                    g = min(G, nkb - kb0)
                    posts = []
                    if gi == ngrp - 1:
                        posts.append(norm_fn)
                        if hh == 1:
                            posts.append(queue_tail)
                    pe_ns = (256 * g + 130 * prev_g[0]) * PE_NS
                    act_ns = (256 * g + 222) * ACT_NS + 32
                    run_unit(lambda kb0=kb0, g=g: s_fn(kb0, g),
                             lambda kb0=kb0, g=g: pv_fn(kb0, g),
                             posts, pe_ns, act_ns)
                    prev_g[0] = g

            # ---- schedule ----
            # st0's own chains are hand-placed around qt0/qt1's units
            # (dispensing one after its consumer would deadlock the
            # in-order PE stream); later tiles' chains flow through the
            # gated filler queues, force-flushed before their first
            # consumer pair.
            for st in (1, 2, 3):
                queue_phaseA(st)

            qk_chain(0, 0)
            qk_chain(0, 2)
            emit_head(0, 0, 0)
            emit_head(0, 0, 1)
            qk_chain(0, 1)
            qk_chain(0, 3)
            v_chain(0, 0)
            v_chain(0, 1)
            emit_head(0, 1, 0)
            emit_head(0, 1, 1)
            emit_head(1, 0, 0)
            v_chain(0, 2)
            v_chain(0, 3)
            emit_head(1, 0, 1)
            emit_head(1, 1, 0)
            emit_head(1, 1, 1)

            for st in (1, 2, 3):
                flush_proj(st)
                for qt in (2 * st, 2 * st + 1):
                    for hp in range(2):
                        for hh in range(2):
                            emit_head(qt, hp, hh)
            flush_pending()
            tail_mode[0] = True
            while late_q:
                _, fn = late_q.pop(0)
                fn()

    nc.compile()
    return nc


def make_in_maps(x, w_qkv, w_out):
    import ml_dtypes
    bf = ml_dtypes.bfloat16
    in_maps = []
    for c in range(N_CORES):
        b, g = divmod(c, 4)
        wq = w_qkv[g * OLOC:(g + 1) * OLOC, :]
        wk = w_qkv[D_MODEL + g * OLOC:D_MODEL + (g + 1) * OLOC, :]
        wvs = w_qkv[2 * D_MODEL + g * OLOC:2 * D_MODEL + (g + 1) * OLOC, :]
        in_maps.append({
            "x_t": np.ascontiguousarray(x[b].T).astype(bf),
            "wqk_t": np.ascontiguousarray(
                np.concatenate([wq, wk], axis=0).T).astype(bf),
            "wv_t": np.ascontiguousarray(wvs.T).astype(bf),
            "wo_t": np.ascontiguousarray(
                w_out[:, g * OLOC:(g + 1) * OLOC].T).astype(bf),
        })
    return in_maps


def kernel(x, w_qkv, w_out):
    x = np.asarray(x, dtype=np.float32)
    w_qkv = np.asarray(w_qkv, dtype=np.float32)
    w_out = np.asarray(w_out, dtype=np.float32)

    if "nc" not in _CACHE:
        _CACHE["nc"] = build_nc()
    nc = _CACHE["nc"]

    in_maps = make_in_maps(x, w_qkv, w_out)
    _CACHE["in_maps"] = in_maps

    res = run_bass_kernel_spmd(nc, in_maps, list(range(N_CORES)))
    out = np.zeros((B, S, D_MODEL), dtype=np.float32)
    for c in range(N_CORES):
        out[c // 4] += np.asarray(res.results[c]["out"], dtype=np.float32)
    return out


# revision 26
# speedup vs baseline: 19668.8648x; 1.0656x over previous
"""Causal multi-head self-attention on 8 TRN2 NeuronCores.

Sharding: batch (2) x head-groups (4) -> 8 cores. Each core computes the
qkv projection for its 4 heads of its batch, full causal attention for
those heads, and a partial output projection (its head slice of w_out);
the host sums the 4 partials per batch.

Per-core pipeline (v3):
  A) x^T arrives pre-transposed (bf16) from the host, so no PE
     transposes. Q^T/K^T (f32r in SBUF, head dims on partitions) via
     w^T-stationary bf16 matmuls; V natural ([s, d]) bf16 with a ones
     column per head (softmax denominator rides the PV matmul).
  B) attention in 256-wide q tiles: S^T = K^T.T @ Q^T (k on partitions,
     f32r), P = exp(S/8) on ACT into bf16, staircase mask on the two
     diagonal k-blocks (DVE), then O[q, d] += P_slice.T @ [V|1] with P as
     the 128-wide bf16 stationary. O is q-on-partitions, so the softmax
     denominator is a per-partition scalar: DVE reciprocal +
     tensor_scalar_mul, fused with the PSUM->SBUF bf16 copy. Two heads
     pack into one [128, 128] bf16 tile, PE-transposed into
     aoT2[hp] = [2 heads' dims, s].
  C) partial[s, :] = sum_hp aoT2[hp].T @ wo2[hp], staged bf16 and DMA'd;
     host converts/sums partials in f32.

Emission is software-pipelined and deficit-paced: each group's PV is one
unit late (PE runs ahead of ACT's exp); projection chains for s-tile
st+1, deferred O-transposes, and output-projection chunks are queued as
PE fillers and dispensed whenever the running ACT-vs-PE budget goes
negative, so PE never starves while ACT crunches exp. Leftover fillers
(mostly phase C) drain at the end, covering ACT's causal-tail overrun.
"""

import math
import numpy as np

import concourse.bacc as bacc
import concourse.mybir as mybir
import concourse.tile as tile
from concourse.masks import make_identity
from concourse.bass_utils import run_bass_kernel_spmd

F32 = mybir.dt.float32
F32R = mybir.dt.float32r
BF16 = mybir.dt.bfloat16
EXP = mybir.ActivationFunctionType.Exp

D_MODEL = 1024
HEAD_DIM = 64
B, S = 2, 2048
N_CORES = 8
OLOC = 256                  # 4 heads x 64 dims per core
SCALE = 1.0 / math.sqrt(HEAD_DIM)
G = 8                       # k-blocks (128 each) per S/exp group

PE_NS = 1.0 / 2.4           # ns per PE cycle at full p-state
ACT_NS = 1.0 / 1.2

_CACHE = {}


def build_nc():
    nc = bacc.Bacc("TRN2", target_bir_lowering=False, debug=False)

    x_d = nc.dram_tensor("x_t", [D_MODEL, S], BF16, kind="ExternalInput")
    wqk_d = nc.dram_tensor("wqk_t", [D_MODEL, 512], BF16, kind="ExternalInput")
    wv_d = nc.dram_tensor("wv_t", [D_MODEL, OLOC], BF16, kind="ExternalInput")
    wo_d = nc.dram_tensor("wo_t", [OLOC, D_MODEL], BF16, kind="ExternalInput")
    out_d = nc.dram_tensor("out", [S, D_MODEL], BF16, kind="ExternalOutput")

    with tile.TileContext(nc) as tc:
        with (
            tc.tile_pool(name="persist", bufs=1) as pp,
            tc.tile_pool(name="work", bufs=2) as wp,
            tc.tile_pool(name="psum", bufs=1, space="PSUM") as psp,
        ):
            ident = pp.tile([128, 128], BF16)
            make_identity(nc, ident[:])

            # staircase causal mask for the 2-block diagonal band of a
            # 256-wide q tile: M[p, c] = 1 iff p <= c - 128. Slice
            # [:, (1-j)*128:][:256] masks diagonal sub-block j.
            mask = pp.tile([128, 384], BF16)
            nc.gpsimd.memset(mask[:], 1.0)
            nc.gpsimd.affine_select(
                out=mask[:], in_=mask[:],
                compare_op=mybir.AluOpType.is_ge,
                fill=0.0, base=-128,
                pattern=[[1, 384]], channel_multiplier=-1,
            )

            xT = pp.tile([128, 8, S], BF16)
            wqk = pp.tile([128, 8, 512], BF16)
            wv = pp.tile([128, 8, OLOC], BF16)
            wo2 = pp.tile([128, 2, D_MODEL], BF16)
            qkT = [pp.tile([128, S], BF16, name=f"qkT{i}") for i in range(4)]
            v_sb = [pp.tile([128, 4, 65], BF16, name=f"v{j}")
                    for j in range(S // 128)]
            aoT2 = [pp.tile([128, S], BF16, name=f"aoT{i}") for i in range(2)]

            for j in range(S // 128):
                nc.gpsimd.memset(v_sb[j][:, :, 64:65], 1.0)

            # input DMAs (all SP queue), ordered so the first s-tile's
            # operands land first: wqk halves + st0 x slices, then wv,
            # then coarser x chunks for s 512.. plus wo.
            nc.sync.dma_start(wqk[:, 0, :], wqk_d[0:128, :])
            nc.sync.dma_start(xT[:, 0, 0:512], x_d[0:128, 0:512])
            nc.sync.dma_start(
                wqk[:, 1:4, :],
                wqk_d[128:512, :].rearrange("(c p) o -> p c o", p=128))
            nc.sync.dma_start(xT[:, 1, 0:512], x_d[128:256, 0:512])
            nc.sync.dma_start(
                wqk[:, 4:8, :],
                wqk_d[512:1024, :].rearrange("(c p) o -> p c o", p=128))
            for it in range(2, 8):
                nc.sync.dma_start(
                    xT[:, it, 0:512],
                    x_d[it * 128:(it + 1) * 128, 0:512])
            nc.sync.dma_start(
                wv[:],
                wv_d[:, :].rearrange("(c p) o -> p c o", p=128))
            for it in range(8):
                nc.sync.dma_start(
                    xT[:, it, 512:2048],
                    x_d[it * 128:(it + 1) * 128, 512:2048])
            nc.sync.dma_start(
                wo2[:],
                wo_d[:, :].rearrange("(hp p) o -> p hp o", p=128))

            # ---- pacing scheduler ----
            # Attention units stream continuously (their real pace is set
            # at runtime by ACT's exp throughput via the 2-deep pst
            # rotation); projection chains for later s-tiles and the
            # deferred transpose / output-projection work are dispensed
            # between units whenever the running (PE - ACT) bank dips
            # below a margin, so PE always has work while ACT crunches
            # and ACT never waits on a long dense PE block. Gates keep a
            # chain from being emitted before its x DMA plausibly landed
            # (a premature chain would block the in-order PE stream).
            sched = {"bank": 3000.0, "ucount": 0}
            MARGIN = 1500.0
            proj_qs = {1: [], 2: [], 3: []}   # st -> chains
            UNIT_GATE = {1: 2, 2: 14, 3: 34}  # min ucount to dispense
            late_q = []          # transposes + phase C (no deadline)

            def next_proj():
                for st in (1, 2, 3):
                    if proj_qs[st] and UNIT_GATE[st] <= sched["ucount"]:
                        return proj_qs[st]
                return None

            def charge(pe_ns, act_ns):
                sched["bank"] += pe_ns - act_ns
                sched["ucount"] += 1
                while sched["bank"] < MARGIN:
                    q = next_proj()
                    if q is None:
                        break
                    pe_ns2, fn = q.pop(0)
                    fn()
                    sched["bank"] += pe_ns2
                # keep phase C in reserve for the ACT-bound causal tail:
                # only draw it down once every projection chain is out
                if not (proj_qs[1] or proj_qs[2] or proj_qs[3]):
                    while sched["bank"] < 0.0 and late_q:
                        pe_ns2, fn = late_q.pop(0)
                        fn()
                        sched["bank"] += pe_ns2

            def flush_proj(st):
                while proj_qs[st]:
                    _, fn = proj_qs[st].pop(0)
                    fn()

            # ---- phase A: Q^T/K^T + V for one 512-wide s tile ----
            def qk_chain(st, ob):
                pqk = psp.tile([128, 512], F32, tag="mm", bufs=2, name="pqk")
                for it in range(8):
                    nc.tensor.matmul(
                        pqk[:],
                        wqk[:, it, ob * 128:(ob + 1) * 128],
                        xT[:, it, st * 512:(st + 1) * 512],
                        start=(it == 0), stop=(it == 7),
                        skip_group_check=True)
                nc.vector.tensor_copy(
                    qkT[ob][:, st * 512:(st + 1) * 512], pqk[:])

            def v_chain(st, j):
                pv = psp.tile([128, OLOC], F32, tag="mm", bufs=2, name="pv")
                s0 = st * 512 + j * 128
                for it in range(8):
                    nc.tensor.matmul(
                        pv[:],
                        xT[:, it, s0:s0 + 128],
                        wv[:, it, :],
                        start=(it == 0), stop=(it == 7),
                        skip_group_check=True)
                vt = v_sb[st * 4 + j]
                nc.vector.tensor_copy(
                    vt[:, :, 0:64],
                    pv[:].rearrange("p (h d) -> p h d", h=4))

            def queue_phaseA(st):
                for ob in range(4):
                    proj_qs[st].append((8 * 512 * PE_NS,
                                        lambda st=st, ob=ob:
                                        qk_chain(st, ob)))
                for j in range(4):
                    proj_qs[st].append((8 * 256 * PE_NS,
                                        lambda st=st, j=j: v_chain(st, j)))

            # ---- phase C: one 128-row output chunk ----
            osb_tiles = {}

            tail_mode = [False]   # end flush: ACT is idle, use it for copies

            def c_chunk(qtile, ob):
                s0 = qtile * 128
                pout = psp.tile([128, 512], F32, tag="mm", bufs=2,
                                name="pout")
                for hp in range(2):
                    nc.tensor.matmul(
                        pout[:],
                        aoT2[hp][:, s0:s0 + 128],
                        wo2[:, hp, ob * 512:(ob + 1) * 512],
                        start=(hp == 0), stop=(hp == 1),
                        skip_group_check=True)
                if ob == 0:
                    osb_tiles[qtile] = wp.tile([128, 2, 512], BF16,
                                               tag="osb", bufs=3,
                                               name="osb")
                osb = osb_tiles[qtile]
                if tail_mode[0] and ob == 0:
                    nc.scalar.copy(osb[:, ob, :], pout[:])
                else:
                    nc.vector.tensor_copy(osb[:, ob, :], pout[:])
                if ob == 1:
                    nc.sync.dma_start(out_d[s0:s0 + 128, :],
                                      osb_tiles.pop(qtile)[:])

            # ---- phase B: attention with lag-2 PV emission ----
            # PV of group g is emitted two units late so the exp+mask
            # chain latency (~1us across ACT/DVE) is fully hidden behind
            # later S groups and dispensed filler work.
            pending = []

            def run_unit(s_fn, pv_fn, posts, pe_ns, act_ns):
                s_fn()
                charge(pe_ns, act_ns)
                pending.append((pv_fn, posts))
                if len(pending) > 2:
                    pv, ps = pending.pop(0)
                    pv()
                    for p in ps:
                        p()

            def flush_pending():
                while pending:
                    pv, ps = pending.pop(0)
                    pv()
                    for p in ps:
                        p()

            o2_tiles = {}

            def emit_head(qt, hp, hh):
                h = 2 * hp + hh
                r0 = (h % 2) * 64
                q_t = qkT[h // 2]
                k_t = qkT[2 + h // 2]
                for qtile in (2 * qt, 2 * qt + 1):
                    nkb = qtile + 1
                    ngrp = (nkb + G - 1) // G
                    state = {}

                    def s_fn(kb0, g, qtile=qtile, nkb=nkb, state=state):
                        pst = psp.tile([128, G, 128], F32, tag="pst",
                                       bufs=2, name="pst")
                        for u in range(g):
                            kb = kb0 + u
                            nc.tensor.matmul(
                                pst[:, u, :],
                                k_t[r0:r0 + 64, kb * 128:(kb + 1) * 128],
                                q_t[r0:r0 + 64,
                                    qtile * 128:(qtile + 1) * 128],
                                start=True, stop=True)
                        p_t = wp.tile([128, G, 128], BF16, tag="p_t",
                                      bufs=4, name="p_t")
                        nc.scalar.activation(p_t[:, 0:g, :], pst[:, 0:g, :],
                                             EXP, scale=SCALE)
                        if kb0 + g == nkb:  # diagonal block: triangle mask
                            nc.vector.tensor_mul(
                                p_t[:, g - 1, :], p_t[:, g - 1, :],
                                mask[:, 128:256])
                        state["p_t", kb0] = p_t

                    def pv_fn(kb0, g, qtile=qtile, nkb=nkb, state=state):
                        if kb0 == 0:
                            state["po"] = psp.tile([128, 65], F32, tag="po",
                                                   bufs=2, name="po")
                        po = state["po"]
                        p_t = state.pop(("p_t", kb0))
                        for u in range(g):
                            kb = kb0 + u
                            nc.tensor.matmul(
                                po[:],
                                p_t[:, u, :],
                                v_sb[kb][:, h, :],
                                start=(kb == 0), stop=(kb == nkb - 1),
                                skip_group_check=True)

                    def norm_fn(qtile=qtile, state=state):
                        po = state["po"]
                        if hh == 0:
                            o2_tiles[qtile, hp] = wp.tile(
                                [128, 128], BF16, tag="o2", bufs=16,
                                name="o2")
                        o2 = o2_tiles[qtile, hp]
                        recip = wp.tile([128, 1], F32, tag="recip", bufs=2,
                                        name="recip")
                        nc.vector.reciprocal(recip[:], po[:, 64:65])
                        nc.vector.tensor_scalar_mul(
                            o2[:, hh * 64:(hh + 1) * 64],
                            po[:, 0:64], recip[:])

                    def trans_fn(qtile=qtile):
                        o2 = o2_tiles.pop((qtile, hp))
                        ptr = psp.tile([128, 128], BF16, tag="mm", bufs=2,
                                       name="ptr")
                        nc.tensor.matmul(ptr[:], o2[:], ident[:],
                                         is_transpose=True,
                                         skip_group_check=True)
                        nc.vector.tensor_copy(
                            aoT2[hp][:, qtile * 128:(qtile + 1) * 128],
                            ptr[:])

                    def queue_tail(qtile=qtile, trans_fn=trans_fn):
                        late_q.append((128 * PE_NS, trans_fn))
                        if hp == 1:
                            for ob in range(2):
                                late_q.append(
                                    (2 * 512 * PE_NS,
                                     lambda qtile=qtile, ob=ob:
                                     c_chunk(qtile, ob)))

                    prev_g = 0
                    for gi in range(ngrp):
                        kb0 = gi * G
                        g = min(G, nkb - kb0)
                        posts = []
                        if gi == ngrp - 1:
                            posts.append(norm_fn)
                            if hh == 1:
                                posts.append(queue_tail)
                        pe_ns = (128 * g + 65 * prev_g) * PE_NS
                        act_ns = (128 * g + 222) * ACT_NS + 32
                        run_unit(lambda kb0=kb0, g=g, f=s_fn: f(kb0, g),
                                 lambda kb0=kb0, g=g, f=pv_fn: f(kb0, g),
                                 posts, pe_ns, act_ns)
                        prev_g = g

            # ---- schedule ----
            # st0's own chains are hand-placed around qt0/qt1's units
            # (dispensing one after its consumer would deadlock the
            # in-order PE stream); later tiles' chains flow through the
            # gated filler queues, force-flushed before their first
            # consumer pair.
            for st in (1, 2, 3):
                queue_phaseA(st)

            qk_chain(0, 0)
            qk_chain(0, 2)
            v_chain(0, 0)
            v_chain(0, 1)
            emit_head(0, 0, 0)
            emit_head(0, 0, 1)
            qk_chain(0, 1)
            qk_chain(0, 3)
            v_chain(0, 2)
            v_chain(0, 3)
            emit_head(0, 1, 0)
            emit_head(0, 1, 1)
            emit_head(1, 0, 0)
            emit_head(1, 0, 1)
            emit_head(1, 1, 0)
            emit_head(1, 1, 1)

            for st in (1, 2, 3):
                flush_proj(st)
                for qt in (2 * st, 2 * st + 1):
                    for hp in range(2):
                        for hh in range(2):
                            emit_head(qt, hp, hh)
            flush_pending()
            tail_mode[0] = True
            while late_q:
                _, fn = late_q.pop(0)
                fn()

    nc.compile()
    return nc


def make_in_maps(x, w_qkv, w_out):
    import ml_dtypes
    bf = ml_dtypes.bfloat16
    in_maps = []
    for c in range(N_CORES):
        b, g = divmod(c, 4)
        wq = w_qkv[g * OLOC:(g + 1) * OLOC, :]
        wk = w_qkv[D_MODEL + g * OLOC:D_MODEL + (g + 1) * OLOC, :]
        wvs = w_qkv[2 * D_MODEL + g * OLOC:2 * D_MODEL + (g + 1) * OLOC, :]
        in_maps.append({
            "x_t": np.ascontiguousarray(x[b].T).astype(bf),
            "wqk_t": np.ascontiguousarray(
                np.concatenate([wq, wk], axis=0).T).astype(bf),
            "wv_t": np.ascontiguousarray(wvs.T).astype(bf),
            "wo_t": np.ascontiguousarray(
                w_out[:, g * OLOC:(g + 1) * OLOC].T).astype(bf),
        })
    return in_maps


def kernel(x, w_qkv, w_out):
    x = np.asarray(x, dtype=np.float32)
    w_qkv = np.asarray(w_qkv, dtype=np.float32)
    w_out = np.asarray(w_out, dtype=np.float32)

    if "nc" not in _CACHE:
        _CACHE["nc"] = build_nc()
    nc = _CACHE["nc"]

    in_maps = make_in_maps(x, w_qkv, w_out)
    _CACHE["in_maps"] = in_maps

    res = run_bass_kernel_spmd(nc, in_maps, list(range(N_CORES)))
    out = np.zeros((B, S, D_MODEL), dtype=np.float32)
    for c in range(N_CORES):
        out[c // 4] += np.asarray(res.results[c]["out"], dtype=np.float32)
    return out
